# revision 7
# baseline (speedup 1.0000x reference)
"""CosineEmbeddingLoss-style kernel for Trainium2 (Bass/Tile), 8-core data parallel.

reference semantics (fp32):
    dot   = sum(x*y, -1); xx = sum(x*x, -1); yy = sum(y*y, -1)
    d     = dot / max(sqrt(xx*yy), EPS)
    per   = where(p == 1, 1 - d, max(0, d - MARGIN))
    loss  = sum(per)

Sharding: rows (N) split contiguously across 8 cores; each core returns its
(128,1) f32 partition partials; host sums them.

Per-core schedule: host interleaves x and y chunk-by-chunk into one DRAM
tensor so each chunk is a single dma_start (x and y of a chunk land together).
Chunks are DMA'd p-major (each SBUF partition holds s consecutive rows → large
contiguous HBM reads). Per 128-row group: dot via DVE scalar_tensor_tensor
(+accum); squares split between ScalarE activation(Square, accum) and DVE to
balance engine busy time. Small trailing chunks shrink the post-DMA straggle.
"""

import numpy as np

import concourse.bacc as bacc
import concourse.tile as tile
from concourse import mybir
from concourse.bass_utils import run_bass_kernel_spmd

N, D = 32768, 1024
N_CORES = 8
ROWS_PER_CORE = N // N_CORES  # 4096
P = 128
CHUNKS = (256, 512, 512, 512, 512, 512, 512, 512, 128, 128)  # rows per dma_start
MARGIN = 0.5
EPS = 1e-8

F32 = mybir.dt.float32
U8 = mybir.dt.uint8
Alu = mybir.AluOpType
Act = mybir.ActivationFunctionType

assert sum(CHUNKS) == ROWS_PER_CORE


def _col_row_map(chunks=CHUNKS):
    """col_rows[p, k] = local row index feeding stats column k at partition p."""
    n_cols = sum(R // P for R in chunks)
    col_rows = np.empty((P, n_cols), dtype=np.int64)
    k = 0
    r0 = 0
    for R in chunks:
        s_count = R // P
        for s in range(s_count):
            col_rows[:, k] = r0 + np.arange(P) * s_count + s
            k += 1
        r0 += R
    return col_rows


def build(d=D, chunks=CHUNKS):
    n_tiles = sum(R // P for R in chunks)
    rows_per_core = sum(chunks)
    max_s = max(R // P for R in chunks)

    nc = bacc.Bacc(
        "TRN2",
        target_bir_lowering=False,
        debug=False,
        enable_asserts=False,
        num_devices=N_CORES,
    )
    xy_dram = nc.dram_tensor("xy", [2 * rows_per_core, d], F32, kind="ExternalInput")
    m_dram = nc.dram_tensor("m", [P, n_tiles], U8, kind="ExternalInput")
    o_dram = nc.dram_tensor("out", [P, 1], F32, kind="ExternalOutput")

    with tile.TileContext(nc) as tc:
        with (
            tc.tile_pool(name="xyin", bufs=3) as xypool,
            tc.tile_pool(name="scratch", bufs=2) as spool,
            tc.tile_pool(name="stats", bufs=1) as statpool,
            tc.tile_pool(name="ep", bufs=1) as eppool,
        ):
            dot_s = statpool.tile([P, n_tiles], F32)
            xx_s = statpool.tile([P, n_tiles], F32)
            yy_s = statpool.tile([P, n_tiles], F32)
            mask_t = statpool.tile([P, n_tiles], U8)
            zero_t = statpool.tile([P, 1], F32)
            negm_t = statpool.tile([P, 1], F32)
            dummy_t = statpool.tile([P, 1], F32)
            nc.vector.memset(zero_t, 0.0)
            nc.vector.memset(negm_t, -MARGIN)
            # First ACT op is a Sqrt so bacc loads the sqrt_and_others table
            # set once; Square/Relu/Copy/Identity are all in that set too.
            nc.scalar.activation(dummy_t, zero_t, Act.Sqrt, bias=zero_t)

            xyap = xy_dram.ap()
            r0 = 0
            t = 0
            for R in chunks:
                s_count = R // P
                xy_t = xypool.tile([P, 2, max_s, d], F32, tag="xy")
                nc.sync.dma_start(
                    out=xy_t[:, :, :s_count, :],
                    in_=xyap[2 * r0 : 2 * r0 + 2 * R, :].rearrange(
                        "(w p s) d -> p w s d", w=2, p=P
                    ),
                )
                for s in range(s_count):
                    x_sl = xy_t[:, 0, s, :]
                    y_sl = xy_t[:, 1, s, :]
                    # dot on DVE
                    prod = spool.tile([P, d], F32, tag="prod")
                    nc.vector.scalar_tensor_tensor(
                        out=prod,
                        in0=x_sl,
                        scalar=1.0,
                        in1=y_sl,
                        op0=Alu.mult,
                        op1=Alu.mult,
                        accum_out=dot_s[:, t : t + 1],
                    )
                    # xx on ACT
                    junkx = spool.tile([P, d], F32, tag="junkx")
                    nc.scalar.activation(
                        out=junkx,
                        in_=x_sl,
                        func=Act.Square,
                        bias=zero_t,
                        accum_out=xx_s[:, t : t + 1],
                    )
                    # yy: half the yy squares go to DVE to balance engine time
                    junky = spool.tile([P, d], F32, tag="junky")
                    if s % 4 >= 2:
                        nc.vector.scalar_tensor_tensor(
                            out=junky,
                            in0=y_sl,
                            scalar=1.0,
                            in1=y_sl,
                            op0=Alu.mult,
                            op1=Alu.mult,
                            accum_out=yy_s[:, t : t + 1],
                        )
                    else:
                        nc.scalar.activation(
                            out=junky,
                            in_=y_sl,
                            func=Act.Square,
                            bias=zero_t,
                            accum_out=yy_s[:, t : t + 1],
                        )
                    t += 1
                r0 += R

            # mask is only needed by the epilogue; don't delay chunk DMAs
            nc.sync.dma_start(out=mask_t, in_=m_dram.ap())

            # ---- epilogue on (P, n_tiles) stats ----
            pr = eppool.tile([P, n_tiles], F32)
            nc.vector.tensor_mul(pr, xx_s, yy_s)
            pr2 = eppool.tile([P, n_tiles], F32)
            nc.vector.tensor_scalar_max(pr2, pr, EPS * EPS)
            s_ = eppool.tile([P, n_tiles], F32)
            nc.scalar.activation(s_, pr2, Act.Sqrt, bias=zero_t)
            rs = eppool.tile([P, n_tiles], F32)
            nc.vector.reciprocal(rs, s_)
            dd = eppool.tile([P, n_tiles], F32)
            nc.vector.tensor_mul(dd, dot_s, rs)
            pos = eppool.tile([P, n_tiles], F32)  # 1 - d
            nc.scalar.activation(pos, dd, Act.Copy, bias=1.0, scale=-1.0)
            neg = eppool.tile([P, n_tiles], F32)  # relu(d - margin)
            nc.scalar.activation(neg, dd, Act.Relu, bias=negm_t)
            per = eppool.tile([P, n_tiles], F32)
            nc.vector.select(per, mask_t, pos, neg)
            row = eppool.tile([P, 1], F32)
            nc.vector.reduce_sum(row, per, axis=mybir.AxisListType.X)
            nc.sync.dma_start(out=o_dram.ap(), in_=row)

    nc.compile()
    return nc


_cached_nc = None


def _get_nc():
    global _cached_nc
    if _cached_nc is None:
        _cached_nc = build()
    return _cached_nc


def _interleave_xy(x_shard, y_shard, d, chunks=CHUNKS):
    rows = x_shard.shape[0]
    xy = np.empty((2 * rows, d), dtype=np.float32)
    r0 = 0
    for R in chunks:
        xy[2 * r0 : 2 * r0 + R] = x_shard[r0 : r0 + R]
        xy[2 * r0 + R : 2 * r0 + 2 * R] = y_shard[r0 : r0 + R]
        r0 += R
    return xy


def _make_in_maps(x, y, p):
    x = np.asarray(x, dtype=np.float32)
    y = np.asarray(y, dtype=np.float32)
    m_full = (np.asarray(p) == 1).astype(np.uint8)
    col_rows = _col_row_map()
    in_maps = []
    for c in range(N_CORES):
        base = c * ROWS_PER_CORE
        sl = slice(base, base + ROWS_PER_CORE)
        in_maps.append(
            {
                "xy": _interleave_xy(x[sl], y[sl], D),
                "m": np.ascontiguousarray(m_full[base + col_rows]),
            }
        )
    return in_maps


def run(x, y, p, trace=False):
    """Returns (loss_scalar_f32, exec_time_ns_or_None)."""
    nc = _get_nc()
    in_maps = _make_in_maps(x, y, p)
    res = run_bass_kernel_spmd(nc, in_maps, list(range(N_CORES)), trace=trace)
    partials = np.stack([r["out"][:, 0] for r in res.results])
    total = np.float32(np.sum(partials, dtype=np.float32))
    return total, res.exec_time_ns


def kernel(x, y, p):
    total, _ = run(x, y, p)
    return total


# revision 8
# speedup vs baseline: 1.2115x; 1.2115x over previous
"""CosineEmbeddingLoss-style kernel for Trainium2 (Bass/Tile), 8-core data parallel.

reference semantics (fp32):
    dot   = sum(x*y, -1); xx = sum(x*x, -1); yy = sum(y*y, -1)
    d     = dot / max(sqrt(xx*yy), EPS)
    per   = where(p == 1, 1 - d, max(0, d - MARGIN))
    loss  = sum(per)

Sharding: rows (N) split contiguously across 8 cores; each core returns its
(128,1) f32 partition partials; host sums them.

Per-core schedule: host interleaves x and y chunk-by-chunk into one DRAM
tensor so each chunk is a single dma_start (x and y of a chunk land together).
Chunks are DMA'd p-major (each SBUF partition holds s consecutive rows → large
contiguous HBM reads). Per 128-row group: dot via DVE scalar_tensor_tensor
(+accum); squares split between ScalarE activation(Square, accum) and DVE to
balance engine busy time. Small trailing chunks shrink the post-DMA straggle.
"""

import numpy as np

import concourse.bacc as bacc
import concourse.tile as tile
from concourse import mybir
from concourse.bass_utils import run_bass_kernel_spmd

N, D = 32768, 1024
N_CORES = 8
ROWS_PER_CORE = N // N_CORES  # 4096
P = 128
CHUNKS = (256, 512, 512, 512, 512, 512, 512, 512, 128, 128)  # rows per dma_start
MARGIN = 0.5
EPS = 1e-8

F32 = mybir.dt.float32
U8 = mybir.dt.uint8
Alu = mybir.AluOpType
Act = mybir.ActivationFunctionType

assert sum(CHUNKS) == ROWS_PER_CORE


def _col_row_map(chunks=CHUNKS):
    """col_rows[p, k] = local row index feeding stats column k at partition p."""
    n_cols = sum(R // P for R in chunks)
    col_rows = np.empty((P, n_cols), dtype=np.int64)
    k = 0
    r0 = 0
    for R in chunks:
        s_count = R // P
        for s in range(s_count):
            col_rows[:, k] = r0 + np.arange(P) * s_count + s
            k += 1
        r0 += R
    return col_rows


def build(d=D, chunks=CHUNKS):
    n_tiles = sum(R // P for R in chunks)
    rows_per_core = sum(chunks)
    max_s = max(R // P for R in chunks)

    nc = bacc.Bacc(
        "TRN2",
        target_bir_lowering=False,
        debug=False,
        enable_asserts=False,
        num_devices=N_CORES,
    )
    xy_dram = nc.dram_tensor("xy", [2 * rows_per_core, d], F32, kind="ExternalInput")
    m_dram = nc.dram_tensor("m", [P, n_tiles], U8, kind="ExternalInput")
    o_dram = nc.dram_tensor("out", [1, 1], F32, kind="ExternalOutput")

    with tile.TileContext(nc) as tc:
        with (
            tc.tile_pool(name="xyin", bufs=3) as xypool,
            tc.tile_pool(name="scratch", bufs=1) as spool,
            tc.tile_pool(name="stats", bufs=1) as statpool,
            tc.tile_pool(name="ep", bufs=1) as eppool,
            tc.tile_pool(name="psum", bufs=1, space="PSUM") as psumpool,
        ):
            dot_s = statpool.tile([P, n_tiles], F32)
            xx_s = statpool.tile([P, n_tiles], F32)
            yy_s = statpool.tile([P, n_tiles], F32)
            mask_t = statpool.tile([P, n_tiles], U8)
            zero_t = statpool.tile([P, 1], F32)
            negm_t = statpool.tile([P, 1], F32)
            dummy_t = statpool.tile([P, 1], F32)
            ones_t = statpool.tile([P, 1], F32)
            # engine-private scratch outputs, reused across iterations
            prod_t = spool.tile([P, D], F32)
            junk_act = spool.tile([P, D], F32)
            junk_dve = spool.tile([P, D], F32)
            nc.vector.memset(ones_t, 1.0)
            nc.vector.memset(zero_t, 0.0)
            nc.vector.memset(negm_t, -MARGIN)
            # First ACT op is a Sqrt so bacc loads the sqrt_and_others table
            # set once; Square/Relu/Copy/Identity are all in that set too.
            nc.scalar.activation(dummy_t, zero_t, Act.Sqrt, bias=zero_t)

            xyap = xy_dram.ap()
            r0 = 0
            t = 0
            for R in chunks:
                s_count = R // P
                xy_t = xypool.tile([P, 2, max_s, d], F32, tag="xy")
                nc.sync.dma_start(
                    out=xy_t[:, :, :s_count, :],
                    in_=xyap[2 * r0 : 2 * r0 + 2 * R, :].rearrange(
                        "(w p s) d -> p w s d", w=2, p=P
                    ),
                )
                for s in range(s_count):
                    x_sl = xy_t[:, 0, s, :]
                    y_sl = xy_t[:, 1, s, :]
                    # dot on DVE
                    nc.vector.scalar_tensor_tensor(
                        out=prod_t,
                        in0=x_sl,
                        scalar=1.0,
                        in1=y_sl,
                        op0=Alu.mult,
                        op1=Alu.mult,
                        accum_out=dot_s[:, t : t + 1],
                    )
                    # xx on ACT
                    nc.scalar.activation(
                        out=junk_act,
                        in_=x_sl,
                        func=Act.Square,
                        bias=zero_t,
                        accum_out=xx_s[:, t : t + 1],
                    )
                    # yy: later groups go to DVE so ACT and DVE finish together
                    if t >= 14:
                        nc.vector.scalar_tensor_tensor(
                            out=junk_dve,
                            in0=y_sl,
                            scalar=1.0,
                            in1=y_sl,
                            op0=Alu.mult,
                            op1=Alu.mult,
                            accum_out=yy_s[:, t : t + 1],
                        )
                    else:
                        nc.scalar.activation(
                            out=junk_act,
                            in_=y_sl,
                            func=Act.Square,
                            bias=zero_t,
                            accum_out=yy_s[:, t : t + 1],
                        )
                    t += 1
                r0 += R

            # mask is only needed by the epilogue; don't delay chunk DMAs
            nc.sync.dma_start(out=mask_t, in_=m_dram.ap())

            # ---- epilogue on (P, n_tiles) stats ----
            pr = eppool.tile([P, n_tiles], F32)
            nc.vector.tensor_mul(pr, xx_s, yy_s)
            pr2 = eppool.tile([P, n_tiles], F32)
            nc.vector.tensor_scalar_max(pr2, pr, EPS * EPS)
            s_ = eppool.tile([P, n_tiles], F32)
            nc.scalar.activation(s_, pr2, Act.Sqrt, bias=zero_t)
            rs = eppool.tile([P, n_tiles], F32)
            nc.vector.reciprocal(rs, s_)
            dd = eppool.tile([P, n_tiles], F32)
            nc.vector.tensor_mul(dd, dot_s, rs)
            pos = eppool.tile([P, n_tiles], F32)  # 1 - d
            nc.scalar.activation(pos, dd, Act.Copy, bias=1.0, scale=-1.0)
            neg = eppool.tile([P, n_tiles], F32)  # relu(d - margin)
            nc.scalar.activation(neg, dd, Act.Relu, bias=negm_t)
            per = eppool.tile([P, n_tiles], F32)
            nc.vector.select(per, mask_t, pos, neg)
            row = eppool.tile([P, 1], F32)
            nc.vector.reduce_sum(row, per, axis=mybir.AxisListType.X)
            ps = psumpool.tile([1, 1], F32)
            nc.tensor.matmul(out=ps, lhsT=row, rhs=ones_t, start=True, stop=True)
            res = eppool.tile([1, 1], F32)
            nc.scalar.copy(res, ps)
            nc.sync.dma_start(out=o_dram.ap(), in_=res)

    nc.compile()
    return nc


_cached_nc = None


def _get_nc():
    global _cached_nc
    if _cached_nc is None:
        _cached_nc = build()
    return _cached_nc


def _interleave_xy(x_shard, y_shard, d, chunks=CHUNKS):
    rows = x_shard.shape[0]
    xy = np.empty((2 * rows, d), dtype=np.float32)
    r0 = 0
    for R in chunks:
        xy[2 * r0 : 2 * r0 + R] = x_shard[r0 : r0 + R]
        xy[2 * r0 + R : 2 * r0 + 2 * R] = y_shard[r0 : r0 + R]
        r0 += R
    return xy


def _make_in_maps(x, y, p):
    x = np.asarray(x, dtype=np.float32)
    y = np.asarray(y, dtype=np.float32)
    m_full = (np.asarray(p) == 1).astype(np.uint8)
    col_rows = _col_row_map()
    in_maps = []
    for c in range(N_CORES):
        base = c * ROWS_PER_CORE
        sl = slice(base, base + ROWS_PER_CORE)
        in_maps.append(
            {
                "xy": _interleave_xy(x[sl], y[sl], D),
                "m": np.ascontiguousarray(m_full[base + col_rows]),
            }
        )
    return in_maps


def run(x, y, p, trace=False):
    """Returns (loss_scalar_f32, exec_time_ns_or_None)."""
    nc = _get_nc()
    in_maps = _make_in_maps(x, y, p)
    res = run_bass_kernel_spmd(nc, in_maps, list(range(N_CORES)), trace=trace)
    partials = np.array([r["out"][0, 0] for r in res.results], dtype=np.float32)
    total = np.float32(np.sum(partials, dtype=np.float32))
    return total, res.exec_time_ns


def kernel(x, y, p):
    total, _ = run(x, y, p)
    return total
